# revision 27
# baseline (speedup 1.0000x reference)
"""AddrNet (vq_codebook) Trainium2 kernel — 8-core pure data parallel.

Reference (per row, DEPTH=8 iterations):
    h = x @ W_in + b_in                       # [B,16]
    loop: logits = h @ W_out + b_out          # [B,256]
          idx = argmax(logits)
          h = silu((h + embed[idx]) @ W_mlp + b_mlp)
Output: stacked idx per depth -> [B, 8] int32.

Kernel design (per core, 65536 rows, 512 chunks of 128):
  - h kept TRANSPOSED in SBUF as hT [17,128] (row 16 = ones, folds biases
    into matmuls as an extra contraction row).
  - logits via one PE matmul: lhsT=hT[17,128], rhs=[W_out;b_out][17,256]
    -> PSUM [128 batch, 256 bins].
  - row max via DVE reduce_max; onehot = is_ge(logits, max) (DVE).
  - onehot transposed via PE (identity matmul) -> PSUM -> ACT copy to SBUF.
  - gather+MLP+argmax-index fused in PE: psum[128,17] accumulates
      hT.T @ [W_mlp|0 ; b_mlp|0]  +  onehotT_lo.T @ [embed_lo@W_mlp | iota_lo]
                                  +  onehotT_hi.T @ [embed_hi@W_mlp | iota_hi]
    col 16 = argmax index as f32 (copied to the int32 output tile).
  - ACT silu -> h, PE transpose -> next hT.
  - depth 0 folds W_in: logits0 = x @ (W_in@W_out) + (b_in@W_out + b_out),
    mlp0 likewise, so only x.T (one PE transpose of the DMA'd chunk) is
    needed and h0 is never materialized.
"""

import numpy as np

import concourse.bass as bass
import concourse.mybir as mybir
import concourse.tile as tile
import concourse.tile_sem_assignment as _tsa
from concourse.bass_utils import run_bass_kernel_spmd

# Keep Tile's default HW-DGE semaphore lane count; _split_sync_waits below
# handles the 1-wait-per-instruction codegen limit.
_tsa.NUM_HWDGE_SEMS = 8

F32 = mybir.dt.float32
F32R = mybir.dt.float32r
BF16 = mybir.dt.bfloat16
I32 = mybir.dt.int32

B, D, HID, NB, DEPTH = 524288, 128, 16, 256, 8
NCORES = 8
BC = B // NCORES          # 65536 rows per core
P = 128                   # rows per chunk
NCHUNK = BC // P          # 512
UNROLL = 64               # chunks per For_i body

USE_F32R = False          # fp32r is reduced precision -> breaks argmax


def _mm_dt(ap):
    return ap.bitcast(F32R) if USE_F32R else ap


def _split_sync_waits(nc, cap=1):
    """Codegen encodes only a few sync-waits per instruction; move excess
    waits onto injected same-engine NoOps right before the instruction
    (engine is in-order, so semantics are preserved)."""
    cnt = 0
    for f in nc.m.functions:
        for b in f.blocks:
            insts = b.instructions
            out_list = []
            for i in insts:
                si = i.sync_info
                if si is not None and si.on_wait and len(si.on_wait) > cap:
                    waits = list(si.on_wait)
                    k = 0
                    while len(waits) > cap:
                        grp, waits = waits[:cap], waits[cap:]
                        nop = mybir.InstNoOp(
                            name=f"{i.name}-wsplit{k}", ins=[], outs=[])
                        nop.engine = i.engine
                        nop.sync_info = mybir.SyncInfo(on_wait=grp, on_update=[])
                        out_list.append(nop)
                        k += 1
                        cnt += 1
                    i.sync_info = mybir.SyncInfo(
                        on_wait=waits, on_update=list(si.on_update))
                out_list.append(i)
            b.instructions = out_list
    return cnt


def build_program(n_chunks=NCHUNK, unroll=UNROLL, use_loop=True, reps=1):
    nc = bass.Bass("TRN2")

    x = nc.dram_tensor("hidden_state", [BC, D], F32, kind="ExternalInput")
    w2out = nc.dram_tensor("w2out", [D, NB], F32, kind="ExternalInput")
    bias2 = nc.dram_tensor("bias2", [1, NB], F32, kind="ExternalInput")
    w2m = nc.dram_tensor("w2m", [D, HID], F32, kind="ExternalInput")
    bias2m = nc.dram_tensor("bias2m", [1, HID], F32, kind="ExternalInput")
    woute = nc.dram_tensor("woute", [33, NB], F32, kind="ExternalInput")
    wmlpe = nc.dram_tensor("wmlpe", [33, HID], F32, kind="ExternalInput")
    e2lo = nc.dram_tensor("e2lo", [128, HID], F32, kind="ExternalInput")
    e2hi = nc.dram_tensor("e2hi", [128, HID], F32, kind="ExternalInput")
    ident = nc.dram_tensor("ident", [128, 128], F32, kind="ExternalInput")
    iotal = nc.dram_tensor("iotal", [128, 1], F32, kind="ExternalInput")
    iotah = nc.dram_tensor("iotah", [128, 1], F32, kind="ExternalInput")
    identb = nc.dram_tensor("identb", [128, 128], BF16, kind="ExternalInput")
    ones1 = nc.dram_tensor("ones1", [1, 128], F32, kind="ExternalInput")
    out = nc.dram_tensor("out", [BC, DEPTH], I32, kind="ExternalOutput")

    x3 = x.rearrange("(n p) d -> n p d", p=P)
    out3 = out.rearrange("(n p) d -> n p d", p=P)

    with tile.TileContext(nc) as tc:
        with (
            tc.tile_pool(name="consts", bufs=1) as cpool,
            tc.tile_pool(name="work", bufs=6) as wpool,
            tc.tile_pool(name="io", bufs=2) as iopool,
            tc.tile_pool(name="hstate", bufs=6) as hpool,
            tc.tile_pool(name="psC", bufs=4, space="PSUM") as psC,
            tc.tile_pool(name="psD", bufs=4, space="PSUM") as psD,
        ):
            # ---- constants (loaded once) ----
            c_w2out = cpool.tile([D, NB], F32)
            c_bias2 = cpool.tile([1, NB], F32)
            c_w2m = cpool.tile([D, HID], F32)
            c_bias2m = cpool.tile([1, HID], F32)
            c_woute = cpool.tile([33, NB], F32)
            c_wmlpe = cpool.tile([33, HID], F32)
            c_e2lo = cpool.tile([128, HID], F32)
            c_e2hi = cpool.tile([128, HID], F32)
            c_iotal = cpool.tile([128, 1], F32)
            c_iotah = cpool.tile([128, 1], F32)
            c_ident = cpool.tile([128, 128], F32)
            c_identb = cpool.tile([128, 128], BF16)
            c_ones1 = cpool.tile([1, 128], F32)
            HSLOTS = 4
            hslots = []
            for _s in range(HSLOTS):
                _h = cpool.tile([33, P], F32, tag=f"hT{_s}")
                nc.vector.memset(_h[0:32, :], 0.0)
                nc.vector.memset(_h[32:33, :], 1.0)
                hslots.append(_h)

            for t, d in (
                (c_w2out, w2out), (c_bias2, bias2), (c_w2m, w2m),
                (c_bias2m, bias2m), (c_woute, woute), (c_wmlpe, wmlpe),
                (c_e2lo, e2lo), (c_e2hi, e2hi), (c_ident, ident),
                (c_iotal, iotal), (c_iotah, iotah),
                (c_identb, identb),
                (c_ones1, ones1),
            ):
                nc.sync.dma_start(out=t, in_=d[:, :])

            GROUP = 4

            def emit_prologue(x_slice, slot):
                pc = psC.tile([P, 136], F32, tag="pc")
                nc.tensor.transpose(pc[:, 0:128], x_slice, c_ident)
                xT = hpool.tile([D, P], F32, tag="xT")
                nc.scalar.activation(xT, pc[:, 0:128],
                                     mybir.ActivationFunctionType.Copy)
                return pc, xT, hslots[slot]

            def emit_depth_group(sts, t):
                """All GROUP chunks' depth-t work, interleaved stage-by-stage
                so each engine's in-order stream alternates between chunks
                instead of stalling on one chunk's chain."""
                G = len(sts)
                pd_ = []
                for k in range(G):
                    pd = psD.tile([P, 512], F32, tag="pd")
                    pd_.append(pd)

                # stage: logits matmul
                for k in range(G):
                    pc, xT, hT = sts[k]
                    L = pd_[k][:, 256:512]
                    if t == 0:
                        nc.tensor.matmul(L, xT, c_w2out, start=True, stop=False)
                        nc.tensor.matmul(L, c_ones1, c_bias2,
                                         start=False, stop=True)
                    else:
                        nc.tensor.matmul(L, hT, c_woute, start=True, stop=True)

                # stage: row max
                mx_ = []
                for k in range(G):
                    mx = wpool.tile([P, 1], F32, tag="mx")
                    nc.vector.tensor_reduce(mx, pd_[k][:, 256:512],
                                            axis=mybir.AxisListType.X,
                                            op=mybir.AluOpType.max)
                    mx_.append(mx)

                # stage: onehot
                oh_ = []
                for k in range(G):
                    oh = wpool.tile([P, NB], BF16, tag="oh")
                    nc.vector.tensor_scalar(oh, pd_[k][:, 256:512], mx_[k], None,
                                            op0=mybir.AluOpType.is_ge)
                    oh_.append(oh)

                # stage: transpose onehot (PE)
                for k in range(G):
                    ohT_p = pd_[k][:, 0:128].bitcast(BF16)
                    nc.tensor.transpose(ohT_p[:, 0:128], oh_[k][:, 0:128],
                                        c_identb)
                    nc.tensor.transpose(ohT_p[:, 128:256], oh_[k][:, 128:256],
                                        c_identb)

                # stage: copy onehotT PSUM->SBUF (ACT)
                ohT_ = []
                for k in range(G):
                    ohT = wpool.tile([128, NB], F32, tag="ohTs")
                    nc.scalar.activation(ohT, pd_[k][:, 0:128].bitcast(BF16),
                                         mybir.ActivationFunctionType.Copy)
                    ohT_.append(ohT)

                # stage: index matmuls + gather/mlp matmuls (PE)
                for k in range(G):
                    pc, xT, hT = sts[k]
                    pidx = pc[:, 128:136]
                    nc.tensor.matmul(pidx[:, t : t + 1], ohT_[k][:, 0:128],
                                     c_iotal, start=True, stop=False)
                    nc.tensor.matmul(pidx[:, t : t + 1], ohT_[k][:, 128:256],
                                     c_iotah, start=False, stop=True)
                    if t < DEPTH - 1:
                        nxt = pd_[k][:, 128:144]
                        if t == 0:
                            nc.tensor.matmul(nxt, xT, c_w2m,
                                             start=True, stop=False)
                            nc.tensor.matmul(nxt, c_ones1, c_bias2m,
                                             start=False, stop=False)
                        else:
                            nc.tensor.matmul(nxt, hT, c_wmlpe,
                                             start=True, stop=False)
                        nc.tensor.matmul(nxt, ohT_[k][:, 0:128], c_e2lo,
                                         start=False, stop=False)
                        nc.tensor.matmul(nxt, ohT_[k][:, 128:256], c_e2hi,
                                         start=False, stop=True)

                if t == DEPTH - 1:
                    return

                # stage: silu (ACT)
                hA_ = []
                for k in range(G):
                    hA = wpool.tile([P, HID], F32, tag="hA")
                    nc.scalar.activation(hA, pd_[k][:, 128:144],
                                         mybir.ActivationFunctionType.Silu)
                    hA_.append(hA)

                # stage: transpose h (PE)
                for k in range(G):
                    nc.tensor.transpose(pd_[k][0:16, 144:272], hA_[k], c_ident)

                # stage: copy hT PSUM->SBUF (ACT)
                for k in range(G):
                    _, _, hT = sts[k]
                    nc.scalar.activation(hT[0:HID, :], pd_[k][0:16, 144:272],
                                         mybir.ActivationFunctionType.Copy)

            def emit_epilogue(outb_slice, st):
                pc, xT, hT = st
                nc.vector.tensor_copy(outb_slice, pc[:, 128:136])

            def emit_group(x_all, outb_all, off):
                sts = [emit_prologue(x_all[:, off + k, :], (off + k) % HSLOTS)
                       for k in range(GROUP)]
                for t in range(DEPTH):
                    emit_depth_group(sts, t)
                for k in range(GROUP):
                    emit_epilogue(outb_all[:, off + k, :], sts[k])

            def emit_body(ci, nch):
                x_all = iopool.tile([P, nch, D], F32, tag="x")
                nc.sync.dma_start(
                    out=x_all,
                    in_=x3[bass.ds(ci, nch), :, :].rearrange("c p d -> p c d"))
                outb_all = iopool.tile([P, nch, DEPTH], I32, tag="outb")
                for g in range(nch // GROUP):
                    emit_group(x_all, outb_all, g * GROUP)
                nc.scalar.dma_start(
                    out=out3[bass.ds(ci, nch), :, :].rearrange("c p d -> p c d"),
                    in_=outb_all)

            if use_loop:
                for _rep in range(reps):
                    with tc.For_i(0, n_chunks, unroll) as ci:
                        emit_body(ci, unroll)
            else:
                for _rep in range(reps):
                    for c0 in range(0, n_chunks, unroll):
                        emit_body(c0, unroll)

    _split_sync_waits(nc)
    return nc


def _folded_params(W_in, b_in, embed, W_mlp, b_mlp, W_out, b_out):
    f64 = np.float64
    W_in, b_in, embed = W_in.astype(f64), b_in.astype(f64), embed.astype(f64)
    W_mlp, b_mlp = W_mlp.astype(f64), b_mlp.astype(f64)
    W_out, b_out = W_out.astype(f64), b_out.astype(f64)

    iota = np.arange(NB, dtype=f64)[:, None]          # [256,1]
    E2 = embed @ W_mlp                                # [256,16]

    p = {
        "w2out": W_in @ W_out,                                    # [128,256]
        "bias2": (b_in @ W_out + b_out)[None, :],                 # [1,256]
        "w2m": W_in @ W_mlp,                                      # [128,16]
        "bias2m": (b_in @ W_mlp + b_mlp)[None, :],                # [1,16]
        "woute": np.concatenate(
            [W_out, np.zeros((16, NB)), b_out[None, :]], axis=0), # [33,256]
        "wmlpe": np.concatenate(
            [W_mlp, np.zeros((16, HID)), b_mlp[None, :]], axis=0), # [33,16]
        "e2lo": E2[0:128],                                        # [128,16]
        "e2hi": E2[128:256],                                      # [128,16]
        "iotal": iota[0:128],                                     # [128,1]
        "iotah": iota[128:256],                                   # [128,1]
        "ident": np.eye(128),
        "identb": np.eye(128),
        "ones1": np.ones((1, 128)),
    }
    import ml_dtypes
    out = {}
    for k, v in p.items():
        dt = ml_dtypes.bfloat16 if k == "identb" else np.float32
        out[k] = np.ascontiguousarray(v, dtype=dt)
    return out


_CACHE = {}


def kernel(hidden_state, W_in, b_in, embed, W_mlp, b_mlp, W_out, b_out):
    hidden_state = np.asarray(hidden_state, dtype=np.float32)
    params = _folded_params(
        np.asarray(W_in), np.asarray(b_in), np.asarray(embed),
        np.asarray(W_mlp), np.asarray(b_mlp),
        np.asarray(W_out), np.asarray(b_out))

    if "nc" not in _CACHE:
        _CACHE["nc"] = build_program()
    nc = _CACHE["nc"]

    in_maps = []
    for c in range(NCORES):
        m = {"hidden_state": np.ascontiguousarray(hidden_state[c * BC:(c + 1) * BC])}
        m.update(params)
        in_maps.append(m)

    res = run_bass_kernel_spmd(nc, in_maps, core_ids=list(range(NCORES)))
    outs = [np.asarray(r["out"]).astype(np.int32) for r in res.results]
    return np.concatenate(outs, axis=0)


if __name__ == "__main__":
    rng = np.random.default_rng(0)
    inputs = {
        "hidden_state": rng.standard_normal((B, D), dtype=np.float32),
        "W_in": rng.uniform(-0.1, 0.1, (D, HID)).astype(np.float32),
        "b_in": rng.uniform(-0.1, 0.1, (HID,)).astype(np.float32),
        "embed": rng.standard_normal((NB, HID), dtype=np.float32),
        "W_mlp": rng.uniform(-0.25, 0.25, (HID, HID)).astype(np.float32),
        "b_mlp": rng.uniform(-0.25, 0.25, (HID,)).astype(np.float32),
        "W_out": rng.uniform(-0.25, 0.25, (HID, NB)).astype(np.float32),
        "b_out": rng.uniform(-0.25, 0.25, (NB,)).astype(np.float32),
    }
    out = kernel(**inputs)
    print(out.shape, out.dtype, out[:4])


# revision 29
# speedup vs baseline: 1.2524x; 1.2524x over previous
"""AddrNet (vq_codebook) Trainium2 kernel — 8-core pure data parallel.

Reference (per row, DEPTH=8 iterations):
    h = x @ W_in + b_in                       # [B,16]
    loop: logits = h @ W_out + b_out          # [B,256]
          idx = argmax(logits)
          h = silu((h + embed[idx]) @ W_mlp + b_mlp)
Output: stacked idx per depth -> [B, 8] int32.

Kernel design (per core, 65536 rows, 512 chunks of 128):
  - h kept TRANSPOSED in SBUF as hT [17,128] (row 16 = ones, folds biases
    into matmuls as an extra contraction row).
  - logits via one PE matmul: lhsT=hT[17,128], rhs=[W_out;b_out][17,256]
    -> PSUM [128 batch, 256 bins].
  - row max via DVE reduce_max; onehot = is_ge(logits, max) (DVE).
  - onehot transposed via PE (identity matmul) -> PSUM -> ACT copy to SBUF.
  - gather+MLP+argmax-index fused in PE: psum[128,17] accumulates
      hT.T @ [W_mlp|0 ; b_mlp|0]  +  onehotT_lo.T @ [embed_lo@W_mlp | iota_lo]
                                  +  onehotT_hi.T @ [embed_hi@W_mlp | iota_hi]
    col 16 = argmax index as f32 (copied to the int32 output tile).
  - ACT silu -> h, PE transpose -> next hT.
  - depth 0 folds W_in: logits0 = x @ (W_in@W_out) + (b_in@W_out + b_out),
    mlp0 likewise, so only x.T (one PE transpose of the DMA'd chunk) is
    needed and h0 is never materialized.
"""

import numpy as np

import concourse.bass as bass
import concourse.mybir as mybir
import concourse.tile as tile
import concourse.tile_sem_assignment as _tsa
from concourse.bass_utils import run_bass_kernel_spmd

# Keep Tile's default HW-DGE semaphore lane count; _split_sync_waits below
# handles the 1-wait-per-instruction codegen limit.
_tsa.NUM_HWDGE_SEMS = 8

F32 = mybir.dt.float32
F32R = mybir.dt.float32r
BF16 = mybir.dt.bfloat16
I32 = mybir.dt.int32

B, D, HID, NB, DEPTH = 524288, 128, 16, 256, 8
NCORES = 8
BC = B // NCORES          # 65536 rows per core
P = 128                   # rows per chunk
NCHUNK = BC // P          # 512
UNROLL = 64               # chunks per For_i body

USE_F32R = False          # fp32r is reduced precision -> breaks argmax

# Ablation switches (profiling only; break correctness when disabled)
ABL = {"mmL": True, "dve": True, "toh": True, "ohcopy": True,
       "mmnxt": True, "actH": True}


def _mm_dt(ap):
    return ap.bitcast(F32R) if USE_F32R else ap


def _split_sync_waits(nc, cap=1):
    """Codegen encodes only a few sync-waits per instruction; move excess
    waits onto injected same-engine NoOps right before the instruction
    (engine is in-order, so semantics are preserved)."""
    cnt = 0
    for f in nc.m.functions:
        for b in f.blocks:
            insts = b.instructions
            out_list = []
            for i in insts:
                si = i.sync_info
                if si is not None and si.on_wait and len(si.on_wait) > cap:
                    waits = list(si.on_wait)
                    k = 0
                    while len(waits) > cap:
                        grp, waits = waits[:cap], waits[cap:]
                        nop = mybir.InstNoOp(
                            name=f"{i.name}-wsplit{k}", ins=[], outs=[])
                        nop.engine = i.engine
                        nop.sync_info = mybir.SyncInfo(on_wait=grp, on_update=[])
                        out_list.append(nop)
                        k += 1
                        cnt += 1
                    i.sync_info = mybir.SyncInfo(
                        on_wait=waits, on_update=list(si.on_update))
                out_list.append(i)
            b.instructions = out_list
    return cnt


def build_program(n_chunks=NCHUNK, unroll=UNROLL, use_loop=True, reps=1):
    nc = bass.Bass("TRN2")

    x = nc.dram_tensor("hidden_state", [BC, D], F32, kind="ExternalInput")
    w2out = nc.dram_tensor("w2out", [D, NB], F32, kind="ExternalInput")
    bias2 = nc.dram_tensor("bias2", [1, NB], F32, kind="ExternalInput")
    w2m = nc.dram_tensor("w2m", [D, HID + 1], F32, kind="ExternalInput")
    bias2m = nc.dram_tensor("bias2m", [1, HID + 1], F32, kind="ExternalInput")
    woute = nc.dram_tensor("woute", [33, NB], F32, kind="ExternalInput")
    wmlpe = nc.dram_tensor("wmlpe", [33, HID + 1], F32, kind="ExternalInput")
    e2lo = nc.dram_tensor("e2lo", [128, HID + 1], F32, kind="ExternalInput")
    e2hi = nc.dram_tensor("e2hi", [128, HID + 1], F32, kind="ExternalInput")
    ident = nc.dram_tensor("ident", [128, 128], F32, kind="ExternalInput")

    identb = nc.dram_tensor("identb", [128, 128], BF16, kind="ExternalInput")
    ones1 = nc.dram_tensor("ones1", [1, 128], F32, kind="ExternalInput")
    out = nc.dram_tensor("out", [BC, DEPTH], I32, kind="ExternalOutput")

    x3 = x.rearrange("(n p) d -> n p d", p=P)
    out3 = out.rearrange("(n p) d -> n p d", p=P)

    with tile.TileContext(nc) as tc:
        with (
            tc.tile_pool(name="consts", bufs=1) as cpool,
            tc.tile_pool(name="work", bufs=6) as wpool,
            tc.tile_pool(name="io", bufs=2) as iopool,
            tc.tile_pool(name="hstate", bufs=6) as hpool,
            tc.tile_pool(name="psC", bufs=2, space="PSUM") as psC,
            tc.tile_pool(name="psD", bufs=6, space="PSUM") as psD,
        ):
            # ---- constants (loaded once) ----
            c_w2out = cpool.tile([D, NB], F32)
            c_bias2 = cpool.tile([1, NB], F32)
            c_w2m = cpool.tile([D, HID + 1], F32)
            c_bias2m = cpool.tile([1, HID + 1], F32)
            c_woute = cpool.tile([33, NB], F32)
            c_wmlpe = cpool.tile([33, HID + 1], F32)
            c_e2lo = cpool.tile([128, HID + 1], F32)
            c_e2hi = cpool.tile([128, HID + 1], F32)
            c_ident = cpool.tile([128, 128], F32)
            c_identb = cpool.tile([128, 128], BF16)
            c_ones1 = cpool.tile([1, 128], F32)
            HSLOTS = 4
            hslots = []
            for _s in range(HSLOTS):
                _h = cpool.tile([33, P], F32, tag=f"hT{_s}")
                nc.vector.memset(_h[0:32, :], 0.0)
                nc.vector.memset(_h[32:33, :], 1.0)
                hslots.append(_h)

            for t, d in (
                (c_w2out, w2out), (c_bias2, bias2), (c_w2m, w2m),
                (c_bias2m, bias2m), (c_woute, woute), (c_wmlpe, wmlpe),
                (c_e2lo, e2lo), (c_e2hi, e2hi), (c_ident, ident),
                (c_identb, identb),
                (c_ones1, ones1),
            ):
                nc.sync.dma_start(out=t, in_=d[:, :])

            GROUP = 4

            def emit_prologue(x_slice, slot):
                pc = psC.tile([P, 128], F32, tag="pc")
                nc.tensor.transpose(pc, x_slice, c_ident)
                xT = hpool.tile([D, P], F32, tag="xT")
                nc.scalar.activation(xT, pc,
                                     mybir.ActivationFunctionType.Copy)
                return pc, xT, hslots[slot]

            def emit_depth_group(sts, t, outb_sl_):
                """All GROUP chunks' depth-t work, interleaved stage-by-stage
                so each engine's in-order stream alternates between chunks
                instead of stalling on one chunk's chain."""
                G = len(sts)
                pd_ = []
                for k in range(G):
                    pd = psD.tile([P, 512], F32, tag="pd")
                    pd_.append(pd)

                # stage: logits matmul
                for k in range(G):
                    if not ABL["mmL"]:
                        break
                    pc, xT, hT = sts[k]
                    L = pd_[k][:, 256:512]
                    if t == 0:
                        nc.tensor.matmul(L, xT, c_w2out, start=True, stop=False)
                        nc.tensor.matmul(L, c_ones1, c_bias2,
                                         start=False, stop=True)
                    else:
                        nc.tensor.matmul(L, hT, c_woute, start=True, stop=True)

                # stage: row max
                mx_ = []
                for k in range(G):
                    if not ABL["dve"]:
                        mx_.append(None); continue
                    mx = wpool.tile([P, 1], F32, tag="mx")
                    nc.vector.tensor_reduce(mx, pd_[k][:, 256:512],
                                            axis=mybir.AxisListType.X,
                                            op=mybir.AluOpType.max)
                    mx_.append(mx)

                # stage: onehot
                oh_ = []
                for k in range(G):
                    oh = wpool.tile([P, NB], BF16, tag="oh")
                    if ABL["dve"]:
                        nc.vector.tensor_scalar(oh, pd_[k][:, 256:512], mx_[k],
                                                None, op0=mybir.AluOpType.is_ge)
                    else:
                        nc.vector.memset(oh, 0.0)
                    oh_.append(oh)

                # stage: transpose onehot (PE)
                for k in range(G):
                    if not ABL["toh"]:
                        break
                    ohT_p = pd_[k][:, 0:128].bitcast(BF16)
                    nc.tensor.transpose(ohT_p[:, 0:128], oh_[k][:, 0:128],
                                        c_identb)
                    nc.tensor.transpose(ohT_p[:, 128:256], oh_[k][:, 128:256],
                                        c_identb)

                # stage: copy onehotT PSUM->SBUF (ACT)
                ohT_ = []
                for k in range(G):
                    ohT = wpool.tile([128, NB], F32, tag="ohTs")
                    if ABL["ohcopy"]:
                        nc.scalar.activation(ohT, pd_[k][:, 0:128].bitcast(BF16),
                                             mybir.ActivationFunctionType.Copy)
                    else:
                        nc.vector.memset(ohT, 0.0)
                    ohT_.append(ohT)

                # stage: fused gather + mlp + idx matmuls (PE)
                # nxt cols 0:16 = mlp pre-silu, col 16 = argmax index
                for k in range(G):
                    if not ABL["mmnxt"]:
                        break
                    pc, xT, hT = sts[k]
                    nxt = pd_[k][:, 128:145]
                    if t == 0:
                        nc.tensor.matmul(nxt, xT, c_w2m,
                                         start=True, stop=False)
                        nc.tensor.matmul(nxt, c_ones1, c_bias2m,
                                         start=False, stop=False)
                    else:
                        nc.tensor.matmul(nxt, hT, c_wmlpe,
                                         start=True, stop=False)
                    nc.tensor.matmul(nxt, ohT_[k][:, 0:128], c_e2lo,
                                     start=False, stop=False)
                    nc.tensor.matmul(nxt, ohT_[k][:, 128:256], c_e2hi,
                                     start=False, stop=True)

                # stage: index out (DVE, converts f32->int32)
                for k in range(G):
                    nc.vector.tensor_copy(outb_sl_[k][:, t : t + 1],
                                          pd_[k][:, 144:145])

                if t == DEPTH - 1:
                    return

                # stage: silu (ACT)
                if not ABL["actH"]:
                    return
                hA_ = []
                for k in range(G):
                    hA = wpool.tile([P, HID], F32, tag="hA")
                    nc.scalar.activation(hA, pd_[k][:, 128:144],
                                         mybir.ActivationFunctionType.Silu)
                    hA_.append(hA)

                # stage: transpose h (PE)
                for k in range(G):
                    nc.tensor.transpose(pd_[k][0:16, 148:276], hA_[k], c_ident)

                # stage: copy hT PSUM->SBUF (ACT)
                for k in range(G):
                    _, _, hT = sts[k]
                    nc.scalar.activation(hT[0:HID, :], pd_[k][0:16, 148:276],
                                         mybir.ActivationFunctionType.Copy)

            def emit_group(x_all, outb_all, off):
                sts = [emit_prologue(x_all[:, off + k, :], (off + k) % HSLOTS)
                       for k in range(GROUP)]
                outb_sl_ = [outb_all[:, off + k, :] for k in range(GROUP)]
                for t in range(DEPTH):
                    emit_depth_group(sts, t, outb_sl_)

            def emit_body(ci, nch):
                x_all = iopool.tile([P, nch, D], F32, tag="x")
                nc.sync.dma_start(
                    out=x_all,
                    in_=x3[bass.ds(ci, nch), :, :].rearrange("c p d -> p c d"))
                outb_all = iopool.tile([P, nch, DEPTH], I32, tag="outb")
                for g in range(nch // GROUP):
                    emit_group(x_all, outb_all, g * GROUP)
                nc.scalar.dma_start(
                    out=out3[bass.ds(ci, nch), :, :].rearrange("c p d -> p c d"),
                    in_=outb_all)

            if use_loop:
                for _rep in range(reps):
                    with tc.For_i(0, n_chunks, unroll) as ci:
                        emit_body(ci, unroll)
            else:
                for _rep in range(reps):
                    for c0 in range(0, n_chunks, unroll):
                        emit_body(c0, unroll)

    _split_sync_waits(nc)
    return nc


def _folded_params(W_in, b_in, embed, W_mlp, b_mlp, W_out, b_out):
    f64 = np.float64
    W_in, b_in, embed = W_in.astype(f64), b_in.astype(f64), embed.astype(f64)
    W_mlp, b_mlp = W_mlp.astype(f64), b_mlp.astype(f64)
    W_out, b_out = W_out.astype(f64), b_out.astype(f64)

    iota = np.arange(NB, dtype=f64)[:, None]          # [256,1]
    E2 = embed @ W_mlp                                # [256,16]

    p = {
        "w2out": W_in @ W_out,                                    # [128,256]
        "bias2": (b_in @ W_out + b_out)[None, :],                 # [1,256]
        "w2m": np.concatenate([W_in @ W_mlp,
                               np.zeros((D, 1))], axis=1),        # [128,17]
        "bias2m": np.concatenate([b_in @ W_mlp + b_mlp,
                                  [0.0]])[None, :],               # [1,17]
        "woute": np.concatenate(
            [W_out, np.zeros((16, NB)), b_out[None, :]], axis=0), # [33,256]
        "wmlpe": np.concatenate(
            [np.concatenate([W_mlp, np.zeros((HID, 1))], axis=1),
             np.zeros((16, HID + 1)),
             np.concatenate([b_mlp, [0.0]])[None, :]], axis=0),   # [33,17]
        "e2lo": np.concatenate([E2[0:128], iota[0:128]], axis=1),   # [128,17]
        "e2hi": np.concatenate([E2[128:256], iota[128:256]], axis=1),  # [128,17]
        "ident": np.eye(128),
        "identb": np.eye(128),
        "ones1": np.ones((1, 128)),
    }
    import ml_dtypes
    out = {}
    for k, v in p.items():
        dt = ml_dtypes.bfloat16 if k == "identb" else np.float32
        out[k] = np.ascontiguousarray(v, dtype=dt)
    return out


_CACHE = {}


def kernel(hidden_state, W_in, b_in, embed, W_mlp, b_mlp, W_out, b_out):
    hidden_state = np.asarray(hidden_state, dtype=np.float32)
    params = _folded_params(
        np.asarray(W_in), np.asarray(b_in), np.asarray(embed),
        np.asarray(W_mlp), np.asarray(b_mlp),
        np.asarray(W_out), np.asarray(b_out))

    if "nc" not in _CACHE:
        _CACHE["nc"] = build_program()
    nc = _CACHE["nc"]

    in_maps = []
    for c in range(NCORES):
        m = {"hidden_state": np.ascontiguousarray(hidden_state[c * BC:(c + 1) * BC])}
        m.update(params)
        in_maps.append(m)

    res = run_bass_kernel_spmd(nc, in_maps, core_ids=list(range(NCORES)))
    outs = [np.asarray(r["out"]).astype(np.int32) for r in res.results]
    return np.concatenate(outs, axis=0)


if __name__ == "__main__":
    rng = np.random.default_rng(0)
    inputs = {
        "hidden_state": rng.standard_normal((B, D), dtype=np.float32),
        "W_in": rng.uniform(-0.1, 0.1, (D, HID)).astype(np.float32),
        "b_in": rng.uniform(-0.1, 0.1, (HID,)).astype(np.float32),
        "embed": rng.standard_normal((NB, HID), dtype=np.float32),
        "W_mlp": rng.uniform(-0.25, 0.25, (HID, HID)).astype(np.float32),
        "b_mlp": rng.uniform(-0.25, 0.25, (HID,)).astype(np.float32),
        "W_out": rng.uniform(-0.25, 0.25, (HID, NB)).astype(np.float32),
        "b_out": rng.uniform(-0.25, 0.25, (NB,)).astype(np.float32),
    }
    out = kernel(**inputs)
    print(out.shape, out.dtype, out[:4])


# revision 31
# speedup vs baseline: 1.2750x; 1.0181x over previous
"""AddrNet (vq_codebook) Trainium2 kernel — 8-core pure data parallel.

Reference (per row, DEPTH=8 iterations):
    h = x @ W_in + b_in                       # [B,16]
    loop: logits = h @ W_out + b_out          # [B,256]
          idx = argmax(logits)
          h = silu((h + embed[idx]) @ W_mlp + b_mlp)
Output: stacked idx per depth -> [B, 8] int32.

Kernel design (per core, 65536 rows, 512 chunks of 128):
  - h kept TRANSPOSED in SBUF as hT [17,128] (row 16 = ones, folds biases
    into matmuls as an extra contraction row).
  - logits via one PE matmul: lhsT=hT[17,128], rhs=[W_out;b_out][17,256]
    -> PSUM [128 batch, 256 bins].
  - row max via DVE reduce_max; onehot = is_ge(logits, max) (DVE).
  - onehot transposed via PE (identity matmul) -> PSUM -> ACT copy to SBUF.
  - gather+MLP+argmax-index fused in PE: psum[128,17] accumulates
      hT.T @ [W_mlp|0 ; b_mlp|0]  +  onehotT_lo.T @ [embed_lo@W_mlp | iota_lo]
                                  +  onehotT_hi.T @ [embed_hi@W_mlp | iota_hi]
    col 16 = argmax index as f32 (copied to the int32 output tile).
  - ACT silu -> h, PE transpose -> next hT.
  - depth 0 folds W_in: logits0 = x @ (W_in@W_out) + (b_in@W_out + b_out),
    mlp0 likewise, so only x.T (one PE transpose of the DMA'd chunk) is
    needed and h0 is never materialized.
"""

import numpy as np

import concourse.bass as bass
import concourse.mybir as mybir
import concourse.tile as tile
import concourse.tile_sem_assignment as _tsa
from concourse.bass_utils import run_bass_kernel_spmd

# Keep Tile's default HW-DGE semaphore lane count; _split_sync_waits below
# handles the 1-wait-per-instruction codegen limit.
_tsa.NUM_HWDGE_SEMS = 8

F32 = mybir.dt.float32
F32R = mybir.dt.float32r
BF16 = mybir.dt.bfloat16
I32 = mybir.dt.int32

B, D, HID, NB, DEPTH = 524288, 128, 16, 256, 8
NCORES = 8
BC = B // NCORES          # 65536 rows per core
P = 128                   # rows per chunk
NCHUNK = BC // P          # 512
UNROLL = 64               # chunks per For_i body

USE_F32R = False          # fp32r is reduced precision -> breaks argmax

# Ablation switches (profiling only; break correctness when disabled)
ABL = {"mmL": True, "dve": True, "toh": True, "ohcopy": True,
       "mmnxt": True, "actH": True}


def _mm_dt(ap):
    return ap.bitcast(F32R) if USE_F32R else ap


def _split_sync_waits(nc, cap=1):
    """Codegen encodes only a few sync-waits per instruction; move excess
    waits onto injected same-engine NoOps right before the instruction
    (engine is in-order, so semantics are preserved)."""
    cnt = 0
    for f in nc.m.functions:
        for b in f.blocks:
            insts = b.instructions
            out_list = []
            for i in insts:
                si = i.sync_info
                if si is not None and si.on_wait and len(si.on_wait) > cap:
                    waits = list(si.on_wait)
                    k = 0
                    while len(waits) > cap:
                        grp, waits = waits[:cap], waits[cap:]
                        nop = mybir.InstNoOp(
                            name=f"{i.name}-wsplit{k}", ins=[], outs=[])
                        nop.engine = i.engine
                        nop.sync_info = mybir.SyncInfo(on_wait=grp, on_update=[])
                        out_list.append(nop)
                        k += 1
                        cnt += 1
                    i.sync_info = mybir.SyncInfo(
                        on_wait=waits, on_update=list(si.on_update))
                out_list.append(i)
            b.instructions = out_list
    return cnt


def build_program(n_chunks=NCHUNK, unroll=UNROLL, use_loop=True, reps=1):
    nc = bass.Bass("TRN2")

    x = nc.dram_tensor("hidden_state", [BC, D], F32, kind="ExternalInput")
    w2out = nc.dram_tensor("w2out", [D, NB], F32, kind="ExternalInput")
    bias2 = nc.dram_tensor("bias2", [1, NB], F32, kind="ExternalInput")
    w2m = nc.dram_tensor("w2m", [D, HID + 1], F32, kind="ExternalInput")
    bias2m = nc.dram_tensor("bias2m", [1, HID + 1], F32, kind="ExternalInput")
    woute = nc.dram_tensor("woute", [33, NB], F32, kind="ExternalInput")
    wmlpe = nc.dram_tensor("wmlpe", [33, HID + 1], F32, kind="ExternalInput")
    e2lo = nc.dram_tensor("e2lo", [128, HID + 1], F32, kind="ExternalInput")
    e2hi = nc.dram_tensor("e2hi", [128, HID + 1], F32, kind="ExternalInput")
    ident = nc.dram_tensor("ident", [128, 128], F32, kind="ExternalInput")

    identb = nc.dram_tensor("identb", [128, 128], BF16, kind="ExternalInput")
    ones1 = nc.dram_tensor("ones1", [1, 128], F32, kind="ExternalInput")
    out = nc.dram_tensor("out", [BC, DEPTH], I32, kind="ExternalOutput")

    x3 = x.rearrange("(n p) d -> n p d", p=P)
    out3 = out.rearrange("(n p) d -> n p d", p=P)

    with tile.TileContext(nc) as tc:
        with (
            tc.tile_pool(name="consts", bufs=1) as cpool,
            tc.tile_pool(name="work", bufs=12) as wpool,
            tc.tile_pool(name="io", bufs=2) as iopool,
            tc.tile_pool(name="hstate", bufs=10) as hpool,
            tc.tile_pool(name="psD", bufs=8, space="PSUM") as psD,
        ):
            # ---- constants (loaded once) ----
            c_w2out = cpool.tile([D, NB], F32)
            c_bias2 = cpool.tile([1, NB], F32)
            c_w2m = cpool.tile([D, HID + 1], F32)
            c_bias2m = cpool.tile([1, HID + 1], F32)
            c_woute = cpool.tile([33, NB], F32)
            c_wmlpe = cpool.tile([33, HID + 1], F32)
            c_e2lo = cpool.tile([128, HID + 1], F32)
            c_e2hi = cpool.tile([128, HID + 1], F32)
            c_ident = cpool.tile([128, 128], F32)
            c_identb = cpool.tile([128, 128], BF16)
            c_ones1 = cpool.tile([1, 128], F32)
            HSLOTS = 8
            hslots = []
            for _s in range(HSLOTS):
                _h = cpool.tile([33, P], F32, tag=f"hT{_s}")
                nc.vector.memset(_h[0:32, :], 0.0)
                nc.vector.memset(_h[32:33, :], 1.0)
                hslots.append(_h)

            for t, d in (
                (c_w2out, w2out), (c_bias2, bias2), (c_w2m, w2m),
                (c_bias2m, bias2m), (c_woute, woute), (c_wmlpe, wmlpe),
                (c_e2lo, e2lo), (c_e2hi, e2hi), (c_ident, ident),
                (c_identb, identb),
                (c_ones1, ones1),
            ):
                nc.sync.dma_start(out=t, in_=d[:, :])

            GROUP = 8

            def emit_prologue(x_slice, slot):
                pcfull = psD.tile([P, 512], F32, tag="pd")
                pc = pcfull[:, 0:128]
                nc.tensor.transpose(pc, x_slice, c_ident)
                xT = hpool.tile([D, P], F32, tag="xT")
                nc.scalar.activation(xT, pc,
                                     mybir.ActivationFunctionType.Copy)
                return pc, xT, hslots[slot]

            def emit_depth_group(sts, t, outb_sl_):
                """All GROUP chunks' depth-t work, interleaved stage-by-stage
                so each engine's in-order stream alternates between chunks
                instead of stalling on one chunk's chain."""
                G = len(sts)
                pd_ = []
                for k in range(G):
                    pd = psD.tile([P, 512], F32, tag="pd")
                    pd_.append(pd)

                # stage: logits matmul
                for k in range(G):
                    if not ABL["mmL"]:
                        break
                    pc, xT, hT = sts[k]
                    L = pd_[k][:, 256:512]
                    if t == 0:
                        nc.tensor.matmul(L, xT, c_w2out, start=True, stop=False)
                        nc.tensor.matmul(L, c_ones1, c_bias2,
                                         start=False, stop=True)
                    else:
                        nc.tensor.matmul(L, hT, c_woute, start=True, stop=True)

                # stage: row max
                mx_ = []
                for k in range(G):
                    if not ABL["dve"]:
                        mx_.append(None); continue
                    mx = wpool.tile([P, 1], F32, tag="mx")
                    nc.vector.tensor_reduce(mx, pd_[k][:, 256:512],
                                            axis=mybir.AxisListType.X,
                                            op=mybir.AluOpType.max)
                    mx_.append(mx)

                # stage: onehot
                oh_ = []
                for k in range(G):
                    oh = wpool.tile([P, NB], BF16, tag="oh")
                    if ABL["dve"]:
                        nc.vector.tensor_scalar(oh, pd_[k][:, 256:512], mx_[k],
                                                None, op0=mybir.AluOpType.is_ge)
                    else:
                        nc.vector.memset(oh, 0.0)
                    oh_.append(oh)

                # stage: transpose onehot (PE)
                for k in range(G):
                    if not ABL["toh"]:
                        break
                    ohT_p = pd_[k][:, 0:128].bitcast(BF16)
                    nc.tensor.transpose(ohT_p[:, 0:128], oh_[k][:, 0:128],
                                        c_identb)
                    nc.tensor.transpose(ohT_p[:, 128:256], oh_[k][:, 128:256],
                                        c_identb)

                # stage: copy onehotT PSUM->SBUF (ACT)
                ohT_ = []
                for k in range(G):
                    ohT = wpool.tile([128, NB], F32, tag="ohTs")
                    if ABL["ohcopy"]:
                        nc.scalar.activation(ohT, pd_[k][:, 0:128].bitcast(BF16),
                                             mybir.ActivationFunctionType.Copy)
                    else:
                        nc.vector.memset(ohT, 0.0)
                    ohT_.append(ohT)

                # stage: fused gather + mlp + idx matmuls (PE)
                # nxt cols 0:16 = mlp pre-silu, col 16 = argmax index
                for k in range(G):
                    if not ABL["mmnxt"]:
                        break
                    pc, xT, hT = sts[k]
                    nxt = pd_[k][:, 128:145]
                    if t == 0:
                        nc.tensor.matmul(nxt, xT, c_w2m,
                                         start=True, stop=False)
                        nc.tensor.matmul(nxt, c_ones1, c_bias2m,
                                         start=False, stop=False)
                    else:
                        nc.tensor.matmul(nxt, hT, c_wmlpe,
                                         start=True, stop=False)
                    nc.tensor.matmul(nxt, ohT_[k][:, 0:128], c_e2lo,
                                     start=False, stop=False)
                    nc.tensor.matmul(nxt, ohT_[k][:, 128:256], c_e2hi,
                                     start=False, stop=True)

                # stage: index out (DVE, converts f32->int32)
                for k in range(G):
                    nc.vector.tensor_copy(outb_sl_[k][:, t : t + 1],
                                          pd_[k][:, 144:145])

                if t == DEPTH - 1:
                    return

                # stage: silu (ACT)
                if not ABL["actH"]:
                    return
                hA_ = []
                for k in range(G):
                    hA = wpool.tile([P, HID], F32, tag="hA")
                    nc.scalar.activation(hA, pd_[k][:, 128:144],
                                         mybir.ActivationFunctionType.Silu)
                    hA_.append(hA)

                # stage: transpose h (PE)
                for k in range(G):
                    nc.tensor.transpose(pd_[k][0:16, 148:276], hA_[k], c_ident)

                # stage: copy hT PSUM->SBUF (ACT)
                for k in range(G):
                    _, _, hT = sts[k]
                    nc.scalar.activation(hT[0:HID, :], pd_[k][0:16, 148:276],
                                         mybir.ActivationFunctionType.Copy)

            def emit_group(x_all, outb_all, off):
                sts = [emit_prologue(x_all[:, off + k, :], (off + k) % HSLOTS)
                       for k in range(GROUP)]
                outb_sl_ = [outb_all[:, off + k, :] for k in range(GROUP)]
                for t in range(DEPTH):
                    emit_depth_group(sts, t, outb_sl_)

            def emit_body(ci, nch):
                x_all = iopool.tile([P, nch, D], F32, tag="x")
                nc.sync.dma_start(
                    out=x_all,
                    in_=x3[bass.ds(ci, nch), :, :].rearrange("c p d -> p c d"))
                outb_all = iopool.tile([P, nch, DEPTH], I32, tag="outb")
                for g in range(nch // GROUP):
                    emit_group(x_all, outb_all, g * GROUP)
                nc.scalar.dma_start(
                    out=out3[bass.ds(ci, nch), :, :].rearrange("c p d -> p c d"),
                    in_=outb_all)

            if use_loop:
                for _rep in range(reps):
                    with tc.For_i(0, n_chunks, unroll) as ci:
                        emit_body(ci, unroll)
            else:
                for _rep in range(reps):
                    for c0 in range(0, n_chunks, unroll):
                        emit_body(c0, unroll)

    _split_sync_waits(nc)
    return nc


def _folded_params(W_in, b_in, embed, W_mlp, b_mlp, W_out, b_out):
    f64 = np.float64
    W_in, b_in, embed = W_in.astype(f64), b_in.astype(f64), embed.astype(f64)
    W_mlp, b_mlp = W_mlp.astype(f64), b_mlp.astype(f64)
    W_out, b_out = W_out.astype(f64), b_out.astype(f64)

    iota = np.arange(NB, dtype=f64)[:, None]          # [256,1]
    E2 = embed @ W_mlp                                # [256,16]

    p = {
        "w2out": W_in @ W_out,                                    # [128,256]
        "bias2": (b_in @ W_out + b_out)[None, :],                 # [1,256]
        "w2m": np.concatenate([W_in @ W_mlp,
                               np.zeros((D, 1))], axis=1),        # [128,17]
        "bias2m": np.concatenate([b_in @ W_mlp + b_mlp,
                                  [0.0]])[None, :],               # [1,17]
        "woute": np.concatenate(
            [W_out, np.zeros((16, NB)), b_out[None, :]], axis=0), # [33,256]
        "wmlpe": np.concatenate(
            [np.concatenate([W_mlp, np.zeros((HID, 1))], axis=1),
             np.zeros((16, HID + 1)),
             np.concatenate([b_mlp, [0.0]])[None, :]], axis=0),   # [33,17]
        "e2lo": np.concatenate([E2[0:128], iota[0:128]], axis=1),   # [128,17]
        "e2hi": np.concatenate([E2[128:256], iota[128:256]], axis=1),  # [128,17]
        "ident": np.eye(128),
        "identb": np.eye(128),
        "ones1": np.ones((1, 128)),
    }
    import ml_dtypes
    out = {}
    for k, v in p.items():
        dt = ml_dtypes.bfloat16 if k == "identb" else np.float32
        out[k] = np.ascontiguousarray(v, dtype=dt)
    return out


_CACHE = {}


def kernel(hidden_state, W_in, b_in, embed, W_mlp, b_mlp, W_out, b_out):
    hidden_state = np.asarray(hidden_state, dtype=np.float32)
    params = _folded_params(
        np.asarray(W_in), np.asarray(b_in), np.asarray(embed),
        np.asarray(W_mlp), np.asarray(b_mlp),
        np.asarray(W_out), np.asarray(b_out))

    if "nc" not in _CACHE:
        _CACHE["nc"] = build_program()
    nc = _CACHE["nc"]

    in_maps = []
    for c in range(NCORES):
        m = {"hidden_state": np.ascontiguousarray(hidden_state[c * BC:(c + 1) * BC])}
        m.update(params)
        in_maps.append(m)

    res = run_bass_kernel_spmd(nc, in_maps, core_ids=list(range(NCORES)))
    outs = [np.asarray(r["out"]).astype(np.int32) for r in res.results]
    return np.concatenate(outs, axis=0)


if __name__ == "__main__":
    rng = np.random.default_rng(0)
    inputs = {
        "hidden_state": rng.standard_normal((B, D), dtype=np.float32),
        "W_in": rng.uniform(-0.1, 0.1, (D, HID)).astype(np.float32),
        "b_in": rng.uniform(-0.1, 0.1, (HID,)).astype(np.float32),
        "embed": rng.standard_normal((NB, HID), dtype=np.float32),
        "W_mlp": rng.uniform(-0.25, 0.25, (HID, HID)).astype(np.float32),
        "b_mlp": rng.uniform(-0.25, 0.25, (HID,)).astype(np.float32),
        "W_out": rng.uniform(-0.25, 0.25, (HID, NB)).astype(np.float32),
        "b_out": rng.uniform(-0.25, 0.25, (NB,)).astype(np.float32),
    }
    out = kernel(**inputs)
    print(out.shape, out.dtype, out[:4])
